# revision 1
# baseline (speedup 1.0000x reference)
"""AUCLoss Trainium2 kernel (8-core SPMD, data-parallel over the sample dim).

Decomposition (validated against the jax reference):
  For ordered pairs (a, b) with ta = target[a] != target[b] = tb:
    M_ab = (proj[a,ta] - proj[b,ta]) / (wn[ta] * dn_ab)
    w_ab = 1 / (counts[tb] * counts[ta]) = cb_b * ra_a
  Per class i the three weighted sums needed are
    W0_i = sum w = exist - 1 (host, exact)
    W1_i = sum w * M
    W2_i = sum w * M^2
  with which loss1/loss2 follow in O(C) on the host.

  On-device per 128-row chunk x 512-col tile (bf16 matmuls, fp32 elsewhere):
    d2   = sq_b - 2 G + sq_a       (PE: K=128 bf16 + K=2 split-sq row;
                                    sq_a via fp32 ACT bias)
    dn   = sqrt(relu-clamped d2)   (ACT; relu only on the diagonal col-tile)
    r    = 1/dn                    (DVE reciprocal_approx_fast)
    t'   = cb * (1-E)(V - u)       (PE: K=93 bf16 hi/lo-split matmul; mask,
                                    weights and -u folded in via onehot rows)
    t''  = sqrt(cb) * (1-E)(V - u) (PE: K=93)
    q    = min(r,C) * t'           (DVE stt, fused row-sum accum -> W1 rows)
    m''  = min(r,C) * t''          (DVE stt)
    W2row = sum m''^2              (ACT Square with accum)
  Row sums [128, 8+8] per chunk are DMA'd out; the tiny per-class scatter
  (ra * onehot) and the final scalar assembly run on the host.

  Columns are rotated per core so each core's own 512 rows (the diagonal
  block of the pair matrix) land in col-tile 0 -- the only tile where
  d2 can round negative and needs the relu clamp.
"""

import numpy as np
from contextlib import ExitStack

import concourse.bass as bass
import concourse.tile as tile
from concourse import bacc, mybir
from concourse.bass_utils import run_bass_kernel_spmd

N = 4096
D = 128
C = 10
NCORES = 8
ROWS = N // NCORES          # 512 rows per core
CHUNKS = ROWS // 128        # 4
FJ = 512                    # col tile width
NJ = N // FJ                # 8
K3 = 3 * C + 1              # 31 logical rows of the t-matmul
KS = 3 * K3                 # 93 after the bf16 hi/lo split
RCLAMP = 1.0   # real pairs have dn >= ~11 (r <= 0.09); only the zero-weight
               # diagonal can produce big r, and clamping it low bounds its
               # rounding-noise contribution at ~1e-8
EPS = 1e-8

f32 = mybir.dt.float32
f16 = mybir.dt.float16
bf16 = mybir.dt.bfloat16
NPBF16 = mybir.dt.np(bf16)

_CACHE = {}

# exposed for test.py
LAST_RESULTS = None


def _build_nc():
    # Bacc (not plain Bass): its compile() runs move_matmul_waits_to_ldweights
    # and generate_event_semaphores -- TRN2 instructions can carry at most one
    # semaphore wait, and Tile freely emits more.
    nc = bacc.Bacc("TRN2", target_bir_lowering=False, debug=False)

    rhs_g = nc.dram_tensor("rhs_g", [D, N], bf16, kind="ExternalInput")
    lhs_g = nc.dram_tensor("lhs_g", [D, ROWS], bf16, kind="ExternalInput")
    sqrow = nc.dram_tensor("sqrow", [2, N], bf16, kind="ExternalInput")
    rhs3a = nc.dram_tensor("rhs3a", [KS, N], bf16, kind="ExternalInput")
    iscb_d = nc.dram_tensor("iscb", [1, N], f16, kind="ExternalInput")
    lhs3 = nc.dram_tensor("lhs3", [KS, ROWS], bf16, kind="ExternalInput")
    biases = nc.dram_tensor("biases", [128, 2 * CHUNKS], f32, kind="ExternalInput")
    out_d = nc.dram_tensor("out", [128, 16 * CHUNKS], f32, kind="ExternalOutput")

    with ExitStack() as ctx:
        tc = ctx.enter_context(tile.TileContext(nc))
        singles = ctx.enter_context(tc.tile_pool(name="singles", bufs=1))
        pd2 = ctx.enter_context(tc.tile_pool(name="pd2", bufs=2, space="PSUM"))
        ptp = ctx.enter_context(tc.tile_pool(name="ptp", bufs=2, space="PSUM"))
        work = ctx.enter_context(tc.tile_pool(name="work", bufs=4))
        stats = ctx.enter_context(tc.tile_pool(name="stats", bufs=4))

        # ---- load inputs ----
        # Critical-path tensors first; strip 0 of every rhs tensor goes on the
        # (fast) sync queue, later strips are split between sync and gpsimd so
        # transfers overlap both each other and early compute.
        NG = 4           # 1024-col groups per chunk row-block
        GW = N // NG     # group width
        sb_lhsg = singles.tile([D, ROWS], bf16)
        sb_l3 = singles.tile([KS, ROWS], bf16)
        sb_bias = singles.tile([128, 2 * CHUNKS], f32)
        sb_sq = singles.tile([2, N], bf16)
        sb_rhsg = [
            singles.tile([D, GW], bf16, tag=f"rhsg{s}", name=f"rhsg{s}")
            for s in range(NG)
        ]
        sb_r3a = [
            singles.tile([KS, GW], bf16, tag=f"r3a{s}", name=f"r3a{s}")
            for s in range(NG)
        ]
        sb_iscb = singles.tile([128, N], f16)
        iscb_bcast = bass.AP(
            tensor=iscb_d.ap().tensor,
            offset=0,
            ap=[[0, 128], [1, N]],
        )
        # group-0 dependencies first, strictly in consumption order
        nc.sync.dma_start(out=sb_lhsg, in_=lhs_g[:, :])
        nc.sync.dma_start(out=sb_rhsg[0], in_=rhs_g[:, 0:GW])
        nc.gpsimd.dma_start(out=sb_l3, in_=lhs3[:, :])
        nc.gpsimd.dma_start(out=sb_r3a[0], in_=rhs3a[:, 0:GW])
        nc.sync.dma_start(out=sb_sq, in_=sqrow[:, :])
        nc.sync.dma_start(out=sb_bias, in_=biases[:, :])
        nc.sync.dma_start(out=sb_rhsg[1], in_=rhs_g[:, GW:2 * GW])
        nc.gpsimd.dma_start(out=sb_r3a[1], in_=rhs3a[:, GW:2 * GW])
        nc.gpsimd.dma_start(out=sb_iscb, in_=iscb_bcast)
        for s in range(2, NG):
            ssl = slice(s * GW, (s + 1) * GW)
            nc.sync.dma_start(out=sb_rhsg[s], in_=rhs_g[:, ssl])
            nc.gpsimd.dma_start(out=sb_r3a[s], in_=rhs3a[:, ssl])
        ones2 = singles.tile([2, 128], bf16)
        nc.vector.memset(ones2, 1.0)
        epsb = singles.tile([128, 1], f32)
        nc.vector.memset(epsb, EPS)

        # W2's Square (ACT) consumes DVE's mpp one group behind, so ACT's
        # Sqrt for the next group is never queued behind a Square that waits
        # on current DVE output.
        sq_backlog = []  # (mpp_tile, w2s_tile, col, chunk)

        def drain_square(n_keep):
            while len(sq_backlog) > n_keep:
                mpps, w2s_t, col, cc = sq_backlog.pop(0)
                junk = work.tile([128, 2 * GW], f16, tag="junk")
                nc.scalar.activation(
                    junk, mpps, mybir.ActivationFunctionType.Square,
                    accum_out=w2s_t[:, col:col + 1],
                )
                if col == NG // 2 - 1:
                    nc.sync.dma_start(
                        out=out_d[:, cc * 16 + 8:cc * 16 + 8 + NG // 2], in_=w2s_t
                    )

        for c in range(CHUNKS):
            csl = slice(c * 128, (c + 1) * 128)
            w1s = stats.tile([128, NG], f32, tag="w1s")
            w2s = stats.tile([128, NG // 2], f32, tag="w2s")
            for g in range(NG):
                d2b = [
                    pd2.tile([128, FJ], f32, tag=f"d2b{h}", name=f"d2b_{c}_{g}_{h}")
                    for h in (0, 1)
                ]
                tp = ptp.tile([128, GW], f32, tag="tp")
                dn2 = work.tile([128, GW], f32, tag="dn")
                for h in (0, 1):
                    nc.tensor.matmul(
                        d2b[h], lhsT=sb_lhsg[:, csl], rhs=sb_rhsg[g][:, h * FJ:(h + 1) * FJ],
                        start=True, stop=False,
                    )
                    nc.tensor.matmul(
                        d2b[h], lhsT=ones2,
                        rhs=sb_sq[:, g * GW + h * FJ:g * GW + (h + 1) * FJ],
                        start=False, stop=True,
                    )
                for h in (0, 1):
                    hs = slice(h * FJ, (h + 1) * FJ)
                    nc.tensor.matmul(
                        tp[:, hs], lhsT=sb_l3[:, csl], rhs=sb_r3a[g][:, hs],
                        start=True, stop=True,
                    )
                for h in (0, 1):
                    # sqrts stay 512-wide: ACT reads PSUM at full rate only
                    # within a single bank
                    hs = slice(h * FJ, (h + 1) * FJ)
                    if g == 0 and h == 0:
                        # diagonal block lives here: clamp d2 before sqrt
                        e = work.tile([128, FJ], f32, tag="e")
                        nc.scalar.activation(
                            e, d2b[h], mybir.ActivationFunctionType.Relu,
                            bias=sb_bias[:, CHUNKS + c:CHUNKS + c + 1],
                            scale=1.0,
                        )
                        nc.scalar.activation(
                            dn2[:, hs], e, mybir.ActivationFunctionType.Sqrt,
                            bias=epsb, scale=1.0,
                        )
                    else:
                        nc.scalar.activation(
                            dn2[:, hs], d2b[h],
                            mybir.ActivationFunctionType.Sqrt,
                            bias=sb_bias[:, c:c + 1], scale=1.0,
                        )
                r2 = work.tile([128, GW], f32, tag="r")
                nc.vector.reciprocal_approx_fast(out=r2, in_=dn2)
                q = work.tile([128, GW], f16, tag="q")
                nc.vector.scalar_tensor_tensor(
                    out=q, in0=r2, scalar=RCLAMP, in1=tp,
                    op0=mybir.AluOpType.min, op1=mybir.AluOpType.mult,
                    accum_out=w1s[:, g:g + 1],
                )
                if g % 2 == 0:
                    mpp2w = work.tile([128, 2 * GW], f16, tag="mpp")
                nc.vector.tensor_tensor(
                    out=mpp2w[:, (g % 2) * GW:(g % 2 + 1) * GW], in0=q,
                    in1=sb_iscb[:, g * GW:(g + 1) * GW],
                    op=mybir.AluOpType.mult,
                )
                if g % 2 == 1:
                    sq_backlog.append((mpp2w, w2s, g // 2, c))
                    drain_square(1)

            nc.sync.dma_start(out=out_d[:, c * 16:c * 16 + NG], in_=w1s)

        drain_square(0)

    nc.compile()
    return nc


def _bf_split(x):
    """x (float64/32) -> (hi, lo) bf16 arrays with hi + lo ~= x."""
    x32 = np.asarray(x, np.float32)
    hi = x32.astype(NPBF16)
    lo = (x32 - hi.astype(np.float32)).astype(NPBF16)
    return hi, lo


def _prep_inputs(pred, target, W):
    pred = np.asarray(pred, dtype=np.float32)
    target = np.asarray(target).astype(np.int64)
    W = np.asarray(W, dtype=np.float32)

    p64 = pred.astype(np.float64)
    sq = (p64 * p64).sum(1)                                   # [N]
    wn = np.maximum(np.sqrt((W.astype(np.float64) ** 2).sum(1)), EPS)
    projn = (p64 @ W.T.astype(np.float64)) / wn[None, :]      # [N, C]
    counts = np.bincount(target, minlength=C)
    cb = 1.0 / np.maximum(counts, 1)[target]                  # [N]
    scb = np.sqrt(cb)
    u = projn[np.arange(N), target]                           # [N]
    onehot = (target[:, None] == np.arange(C)[None, :]).astype(np.float64)

    gh, _ = _bf_split(-2.0 * pred.T)                          # hi only for G
    sqh, sql = _bf_split(sq)
    sqrow_full = np.stack([sqh, sql]).astype(NPBF16)          # [2, N]

    def rows3(colw):
        m = np.empty((K3, N), dtype=np.float64)
        m[0:C] = colw[None, :] * projn.T
        m[C] = -colw
        m[C + 1:2 * C + 1] = -(colw[None, :] * (onehot * projn).T)
        m[2 * C + 1:3 * C + 1] = colw[None, :] * onehot.T
        h, l = _bf_split(m)
        # row pairing: (l3h, h), (l3h, l), (l3l, h)
        return np.concatenate([h, l, h], axis=0)              # [KS, N] bf16

    r3a_full = rows3(cb)
    iscb_full = (1.0 / scb).astype(np.float16)                    # sqrt(counts[target])

    l3_64 = np.empty((K3, N), dtype=np.float64)
    l3_64[0:C] = onehot.T
    l3_64[C] = u
    l3_64[C + 1:2 * C + 1] = onehot.T
    l3_64[2 * C + 1:3 * C + 1] = (u[:, None] * onehot).T
    l3h, l3l = _bf_split(l3_64)
    l3_full = np.concatenate([l3h, l3h, l3l], axis=0)         # [KS, N] bf16

    in_maps = []
    for k in range(NCORES):
        rs = slice(k * ROWS, (k + 1) * ROWS)
        rot = np.roll(np.arange(N), -k * ROWS)                # own rows -> cols 0..511
        sq_own = sq[rs].reshape(CHUNKS, 128).T                # [128, CHUNKS]
        biases = np.concatenate(
            [sq_own + EPS, sq_own - EPS], axis=1
        ).astype(np.float32)                                  # sqrt-bias | relu-bias
        in_maps.append(
            {
                "rhs_g": np.ascontiguousarray(gh[:, rot]),
                "lhs_g": np.ascontiguousarray(pred.T[:, rs].astype(NPBF16)),
                "sqrow": np.ascontiguousarray(sqrow_full[:, rot]),
                "rhs3a": np.ascontiguousarray(r3a_full[:, rot]),
                "iscb": np.ascontiguousarray(iscb_full[rot][None, :]),
                "lhs3": np.ascontiguousarray(l3_full[:, rs]),
                "biases": biases,
            }
        )
    aux = {"counts": counts, "cb": cb, "onehot": onehot}
    return in_maps, aux


def _finish(per_core_out, aux):
    counts, cb, onehot = aux["counts"], aux["cb"], aux["onehot"]
    S = np.zeros((C, 2), dtype=np.float64)
    for k, o in enumerate(per_core_out):
        o = o.astype(np.float64)
        for c in range(CHUNKS):
            rs = slice(k * ROWS + c * 128, k * ROWS + (c + 1) * 128)
            sc = onehot[rs] * cb[rs, None]                    # [128, C]
            S[:, 0] += sc.T @ o[:, c * 16:c * 16 + 8].sum(1)
            S[:, 1] += sc.T @ o[:, c * 16 + 8:c * 16 + 8 + NJ // 2].sum(1)
    exist = float((counts > 0).sum())
    valid = counts > 0
    W1 = -S[:, 0]
    W2 = S[:, 1]
    W0 = exist - 1.0
    denom = exist - 1.0
    l1 = (W0 - 2.0 * W1 + W2) / denom
    mmn = W1 / denom
    mv = (W2 - 2.0 * mmn * W1 + mmn * mmn * W0) / denom
    safe_mm = np.where(mmn == 0.0, 1.0, mmn)
    loss1 = float(np.where(valid, l1, 0.0).sum() / exist)
    loss2 = float(np.where(valid, np.abs(mv / safe_mm), 0.0).sum() / exist)
    return (
        np.asarray(loss1, dtype=np.float32),
        np.asarray(loss2, dtype=np.float32),
    )


def kernel(pred, target, W):
    global LAST_RESULTS
    if "nc" not in _CACHE:
        _CACHE["nc"] = _build_nc()
    nc = _CACHE["nc"]
    in_maps, aux = _prep_inputs(pred, target, W)
    res = run_bass_kernel_spmd(nc, in_maps, list(range(NCORES)))
    LAST_RESULTS = res
    per_core = [res.results[k]["out"] for k in range(NCORES)]
    return _finish(per_core, aux)



# revision 2
# speedup vs baseline: 1.0940x; 1.0940x over previous
"""AUCLoss Trainium2 kernel (8-core SPMD, data-parallel over the sample dim).

Decomposition (validated against the jax reference):
  Samples are pre-sorted by class on the host, so each class occupies a
  contiguous run of columns.  For ordered pairs (a, b), ta = target[a]:
    M_ab = (proj[a,ta] - proj[b,ta]) / (wn[ta] * dn_ab)
    q_ab = min(r_ab, 1) * scb_b * (projn[b,ta] - u_a) * [tb != ta]
  with r = rsqrt(d2 + eps), scb_b = 1/sqrt(counts[tb]).  Then per class i
    W1_i = -sum_{a in i} cb_a * sum_j scb_j * (sum_{b in class j} q_ab)
    W2_i =  sum_{a in i} cb_a * sum_b q_ab^2
  and loss1/loss2 follow in O(C) on the host.

  On-device per 128-row chunk x 1024-col group (bf16 matmuls, f16 elsewhere):
    d2   = sq_b - 2 G            (PE: K=128 bf16 + K=2 split-sq rows)
    r    = Abs_reciprocal_sqrt(d2 + sq_a + EPS_D)   (ACT, one pass; |x|
           absorbs rounding-negative d2, EPS_D keeps the diagonal finite)
    q    = min(r,1) * tp         (DVE stt, per class-column segment, each
           segment's row-sum accumulated -> W1 segment columns)
    w2   = sum q^2               (ACT Square-accum or DVE stt q*q-accum,
           alternating per group to balance the two engines)
  tp = scb_b*(projn[b,ta]-u_a)*mask comes from a K=93 bf16 hi/lo-split
  matmul (onehot rows fold mask, scb weights and -u in).
  Row stats [128, nseg+4] per chunk are DMA'd out; the final per-class
  scatter and scalar assembly run on the host.
"""

import numpy as np
from contextlib import ExitStack

import concourse.bass as bass
import concourse.tile as tile
from concourse import bacc, mybir
from concourse.bass_utils import run_bass_kernel_spmd

N = 4096
D = 128
C = 10
NCORES = 8
ROWS = N // NCORES          # 512 rows per core
CHUNKS = ROWS // 128        # 4
GW = 1024                   # col group width
NG = N // GW                # 4
K3 = 3 * C + 1              # 31 logical rows of the tp-matmul
KS = 3 * K3                 # 93 after the bf16 hi/lo split
RCLAMP = 1.0
EPS_D = 0.01   # rsqrt bias: diagonal d2 cancels to ~0 +- 3e-5 fp noise, so
               # +0.01 keeps r_diag <= ~10 (then min-clamped); off-diagonal
               # d2 >= ~120 so the bias error is <= 4e-5 relative
NWARM = 10     # dummy K=2 matmuls at start to push PE HAM to K=8/8

f32 = mybir.dt.float32
f16 = mybir.dt.float16
bf16 = mybir.dt.bfloat16
NPBF16 = mybir.dt.np(bf16)

_CACHE = {}

# exposed for test.py
LAST_RESULTS = None


def _build_nc(seg_layout):
    """seg_layout: tuple per group of ((start, end, segidx), ...) column
    segments (class runs clipped to the group)."""
    nseg = 1 + max(s[2] for g in seg_layout for s in g)
    ocols = nseg + NG                       # w1 segments | w2 per group
    nc = bacc.Bacc("TRN2", target_bir_lowering=False, debug=False)

    rhs_g = nc.dram_tensor("rhs_g", [D, N], bf16, kind="ExternalInput")
    lhs_g = nc.dram_tensor("lhs_g", [D, ROWS], bf16, kind="ExternalInput")
    sqrow = nc.dram_tensor("sqrow", [2, N], bf16, kind="ExternalInput")
    rhs3a = nc.dram_tensor("rhs3a", [KS, N], bf16, kind="ExternalInput")
    lhs3 = nc.dram_tensor("lhs3", [KS, ROWS], bf16, kind="ExternalInput")
    biases = nc.dram_tensor("biases", [128, CHUNKS], f32, kind="ExternalInput")
    out_d = nc.dram_tensor("out", [128, ocols * CHUNKS], f32, kind="ExternalOutput")

    with ExitStack() as ctx:
        tc = ctx.enter_context(tile.TileContext(nc))
        singles = ctx.enter_context(tc.tile_pool(name="singles", bufs=1))
        pd2 = ctx.enter_context(tc.tile_pool(name="pd2", bufs=2, space="PSUM"))
        ptp = ctx.enter_context(tc.tile_pool(name="ptp", bufs=2, space="PSUM"))
        work = ctx.enter_context(tc.tile_pool(name="work", bufs=4))
        stats = ctx.enter_context(tc.tile_pool(name="stats", bufs=4))

        # ---- load inputs ----
        # group-0 dependencies first, strictly in consumption order; strips
        # split between the sync and gpsimd queues so transfers overlap.
        sb_lhsg = singles.tile([D, ROWS], bf16)
        sb_l3 = singles.tile([KS, ROWS], bf16)
        sb_bias = singles.tile([128, CHUNKS], f32)
        sb_sq = singles.tile([2, N], bf16)
        sb_rhsg = [
            singles.tile([D, GW], bf16, tag=f"rhsg{s}", name=f"rhsg{s}")
            for s in range(NG)
        ]
        sb_r3a = [
            singles.tile([KS, GW], bf16, tag=f"r3a{s}", name=f"r3a{s}")
            for s in range(NG)
        ]
        nc.sync.dma_start(out=sb_lhsg, in_=lhs_g[:, :])
        nc.sync.dma_start(out=sb_rhsg[0], in_=rhs_g[:, 0:GW])
        nc.gpsimd.dma_start(out=sb_l3, in_=lhs3[:, :])
        nc.gpsimd.dma_start(out=sb_r3a[0], in_=rhs3a[:, 0:GW])
        nc.sync.dma_start(out=sb_sq, in_=sqrow[:, :])
        nc.sync.dma_start(out=sb_bias, in_=biases[:, :])
        nc.sync.dma_start(out=sb_rhsg[1], in_=rhs_g[:, GW:2 * GW])
        nc.gpsimd.dma_start(out=sb_r3a[1], in_=rhs3a[:, GW:2 * GW])
        for s in range(2, NG):
            ssl = slice(s * GW, (s + 1) * GW)
            nc.sync.dma_start(out=sb_rhsg[s], in_=rhs_g[:, ssl])
            nc.gpsimd.dma_start(out=sb_r3a[s], in_=rhs3a[:, ssl])
        ones2 = singles.tile([2, 128], bf16)
        nc.vector.memset(ones2, 1.0)
        onesw = singles.tile([2, 512], bf16)
        nc.vector.memset(onesw, 1.0)

        # PE warm-up: HAM starts at K=4/8 (1.2 GHz); ~3.4us of dummy matmul
        # activity during the input DMA wait unthrottles it before the real
        # matmuls start.
        warm = pd2.tile([128, GW], f32, tag="d2", name="warm")
        for _ in range(NWARM):
            nc.tensor.matmul(warm[:, 0:512], lhsT=ones2, rhs=onesw,
                             start=True, stop=True)

        for c in range(CHUNKS):
            csl = slice(c * 128, (c + 1) * 128)
            st = stats.tile([128, ocols], f32, tag="st")
            for g in range(NG):
                d2t = pd2.tile([128, GW], f32, tag="d2", name=f"d2_{c}_{g}")
                tp = ptp.tile([128, GW], f32, tag="tp", name=f"tp_{c}_{g}")
                for h in (0, 1):
                    hs = slice(h * 512, (h + 1) * 512)
                    gs = slice(g * GW + h * 512, g * GW + (h + 1) * 512)
                    nc.tensor.matmul(
                        d2t[:, hs], lhsT=sb_lhsg[:, csl],
                        rhs=sb_rhsg[g][:, hs], start=True, stop=False,
                    )
                    nc.tensor.matmul(
                        d2t[:, hs], lhsT=ones2, rhs=sb_sq[:, gs],
                        start=False, stop=True,
                    )
                for h in (0, 1):
                    hs = slice(h * 512, (h + 1) * 512)
                    nc.tensor.matmul(
                        tp[:, hs], lhsT=sb_l3[:, csl], rhs=sb_r3a[g][:, hs],
                        start=True, stop=True,
                    )
                r = work.tile([128, GW], f16, tag="r")
                for h in (0, 1):
                    # rsqrt stays 512-wide: ACT reads PSUM at full rate only
                    # within a single bank
                    hs = slice(h * 512, (h + 1) * 512)
                    nc.scalar.activation(
                        r[:, hs], d2t[:, hs],
                        mybir.ActivationFunctionType.Abs_reciprocal_sqrt,
                        bias=sb_bias[:, c:c + 1], scale=1.0,
                    )
                q = work.tile([128, GW], f16, tag="q")
                for (s, e, sidx) in seg_layout[g]:
                    ls = slice(s - g * GW, e - g * GW)
                    nc.vector.scalar_tensor_tensor(
                        out=q[:, ls], in0=r[:, ls], scalar=RCLAMP,
                        in1=tp[:, ls], op0=mybir.AluOpType.min,
                        op1=mybir.AluOpType.mult,
                        accum_out=st[:, sidx:sidx + 1],
                    )
                junk = work.tile([128, GW], f16, tag="junk")
                w2col = st[:, nseg + g:nseg + g + 1]
                if (c * NG + g) % 2 == 0:
                    nc.scalar.activation(
                        junk, q, mybir.ActivationFunctionType.Square,
                        accum_out=w2col,
                    )
                else:
                    nc.vector.scalar_tensor_tensor(
                        out=junk, in0=q, scalar=1.0, in1=q,
                        op0=mybir.AluOpType.mult, op1=mybir.AluOpType.mult,
                        accum_out=w2col,
                    )
            nc.sync.dma_start(out=out_d[:, c * ocols:(c + 1) * ocols], in_=st)

    nc.compile()
    return nc, nseg, ocols


def _bf_split(x):
    """x (float64/32) -> (hi, lo) bf16 arrays with hi + lo ~= x."""
    x32 = np.asarray(x, np.float32)
    hi = x32.astype(NPBF16)
    lo = (x32 - hi.astype(np.float32)).astype(NPBF16)
    return hi, lo


def _segments(counts):
    """Class runs in sorted-column order, clipped to GW-wide groups.
    Returns (seg_layout, seg_class): per-group tuples of (s, e, segidx)
    and the class id of each segidx."""
    bounds = np.concatenate([[0], np.cumsum(counts)])
    seg_layout = []
    seg_class = []
    sidx = 0
    for g in range(NG):
        g0, g1 = g * GW, (g + 1) * GW
        segs = []
        for j in range(C):
            s, e = max(bounds[j], g0), min(bounds[j + 1], g1)
            if s < e:
                segs.append((int(s), int(e), sidx))
                seg_class.append(j)
                sidx += 1
        seg_layout.append(tuple(segs))
    return tuple(seg_layout), np.asarray(seg_class)


def _prep_inputs(pred, target, W):
    pred = np.asarray(pred, dtype=np.float32)
    target = np.asarray(target).astype(np.int64)
    W = np.asarray(W, dtype=np.float32)

    perm = np.argsort(target, kind="stable")
    pred = pred[perm]
    target = target[perm]

    p64 = pred.astype(np.float64)
    sq = (p64 * p64).sum(1)                                   # [N]
    wn = np.maximum(np.sqrt((W.astype(np.float64) ** 2).sum(1)), 1e-8)
    projn = (p64 @ W.T.astype(np.float64)) / wn[None, :]      # [N, C]
    counts = np.bincount(target, minlength=C)
    scb = 1.0 / np.sqrt(np.maximum(counts, 1))                # [C]
    colw = scb[target]                                        # [N]
    u = projn[np.arange(N), target]                           # [N]
    onehot = (target[:, None] == np.arange(C)[None, :]).astype(np.float64)

    gh, _ = _bf_split(-2.0 * pred.T)                          # hi only for G
    sqh, sql = _bf_split(sq)
    sqrow_full = np.stack([sqh, sql]).astype(NPBF16)          # [2, N]

    m = np.empty((K3, N), dtype=np.float64)
    m[0:C] = colw[None, :] * projn.T
    m[C] = -colw
    m[C + 1:2 * C + 1] = -(colw[None, :] * (onehot * projn).T)
    m[2 * C + 1:3 * C + 1] = colw[None, :] * onehot.T
    h, l = _bf_split(m)
    # row pairing: (l3h, h), (l3h, l), (l3l, h)
    r3a_full = np.concatenate([h, l, h], axis=0)              # [KS, N] bf16

    l3_64 = np.empty((K3, N), dtype=np.float64)
    l3_64[0:C] = onehot.T
    l3_64[C] = u
    l3_64[C + 1:2 * C + 1] = onehot.T
    l3_64[2 * C + 1:3 * C + 1] = (u[:, None] * onehot).T
    l3h, l3l = _bf_split(l3_64)
    l3_full = np.concatenate([l3h, l3h, l3l], axis=0)         # [KS, N] bf16

    in_maps = []
    for k in range(NCORES):
        rs = slice(k * ROWS, (k + 1) * ROWS)
        sq_own = sq[rs].reshape(CHUNKS, 128).T                # [128, CHUNKS]
        in_maps.append(
            {
                "rhs_g": np.ascontiguousarray(gh),
                "lhs_g": np.ascontiguousarray(pred.T[:, rs].astype(NPBF16)),
                "sqrow": np.ascontiguousarray(sqrow_full),
                "rhs3a": np.ascontiguousarray(r3a_full),
                "lhs3": np.ascontiguousarray(l3_full[:, rs]),
                "biases": (sq_own + EPS_D).astype(np.float32),
            }
        )
    aux = {"counts": counts, "target": target, "scb": scb}
    return in_maps, aux


def _finish(per_core_out, aux, nseg, ocols, seg_class):
    counts, target, scb = aux["counts"], aux["target"], aux["scb"]
    cb = 1.0 / np.maximum(counts, 1)
    cb_a = cb[target]                                         # [N]
    mu = scb[seg_class]                                       # per-segment factor
    S = np.zeros((C, 2), dtype=np.float64)
    for k, o in enumerate(per_core_out):
        o = o.astype(np.float64)
        for c in range(CHUNKS):
            rs = slice(k * ROWS + c * 128, k * ROWS + (c + 1) * 128)
            oc = o[:, c * ocols:(c + 1) * ocols]
            inner1 = oc[:, :nseg] @ mu                        # [128]
            w2row = oc[:, nseg:nseg + NG].sum(1)              # [128]
            wrow = cb_a[rs]                                   # [128]
            tcls = target[rs]
            np.add.at(S[:, 0], tcls, wrow * inner1)
            np.add.at(S[:, 1], tcls, wrow * w2row)
    exist = float((counts > 0).sum())
    valid = counts > 0
    W1 = -S[:, 0]
    W2 = S[:, 1]
    W0 = exist - 1.0
    denom = exist - 1.0
    l1 = (W0 - 2.0 * W1 + W2) / denom
    mmn = W1 / denom
    mv = (W2 - 2.0 * mmn * W1 + mmn * mmn * W0) / denom
    safe_mm = np.where(mmn == 0.0, 1.0, mmn)
    loss1 = float(np.where(valid, l1, 0.0).sum() / exist)
    loss2 = float(np.where(valid, np.abs(mv / safe_mm), 0.0).sum() / exist)
    return (
        np.asarray(loss1, dtype=np.float32),
        np.asarray(loss2, dtype=np.float32),
    )


def kernel(pred, target, W):
    global LAST_RESULTS
    in_maps, aux = _prep_inputs(pred, target, W)
    seg_layout, seg_class = _segments(aux["counts"])
    if seg_layout not in _CACHE:
        _CACHE[seg_layout] = _build_nc(seg_layout)
    nc, nseg, ocols = _CACHE[seg_layout]
    res = run_bass_kernel_spmd(nc, in_maps, list(range(NCORES)))
    LAST_RESULTS = res
    per_core = [res.results[k]["out"] for k in range(NCORES)]
    return _finish(per_core, aux, nseg, ocols, seg_class)


# revision 3
# speedup vs baseline: 1.5157x; 1.3855x over previous
"""AUCLoss Trainium2 kernel (8-core SPMD, data-parallel over the sample dim).

Decomposition (validated against the jax reference):
  Samples are pre-sorted by class on the host, so each class occupies a
  contiguous run of columns.  For ordered pairs (a, b), ta = target[a]:
    M_ab = (proj[a,ta] - proj[b,ta]) / (wn[ta] * dn_ab)
    q_ab = min(r_ab, 1) * scb_b * (projn[b,ta] - u_a) * [tb != ta]
  with r = rsqrt(d2 + eps), scb_b = 1/sqrt(counts[tb]).  Then per class i
    W1_i = -sum_{a in i} cb_a * sum_j scb_j * (sum_{b in class j} q_ab)
    W2_i =  sum_{a in i} cb_a * sum_b q_ab^2
  and loss1/loss2 follow in O(C) on the host.

  On-device per 128-row chunk x 1024-col group (bf16 matmuls, f16 elsewhere):
    d2   = sq_b - 2 G     (PE: K=128 bf16 G-matmul;  the K=2 split-sq rows
           ride at tile_position (32,0) CONCURRENT with the K=31 tp-matmul
           at rows 0-30 -- different PE row-groups execute in parallel, so
           the sq matmuls are ~free)
    r    = Abs_reciprocal_sqrt(d2 + sq_a + EPS_D)   (ACT, one pass; |x|
           absorbs rounding-negative d2, EPS_D keeps the diagonal finite)
    q    = min(r,1) * tp  (DVE stt, per class-column segment, each
           segment's row-sum accumulated -> W1 segment columns)
    w2   = sum q^2        (ACT Square-accum or DVE stt q*q-accum,
           alternating per group to balance the two engines)
  Row stats [128, nseg+4] per chunk are DMA'd out; the final per-class
  scatter and scalar assembly run on the host.
"""

import numpy as np
from contextlib import ExitStack

import concourse.bass as bass
import concourse.tile as tile
from concourse import bacc, mybir
from concourse.bass_utils import run_bass_kernel_spmd

N = 4096
D = 128
C = 10
NCORES = 8
ROWS = N // NCORES          # 512 rows per core
CHUNKS = ROWS // 128        # 4
GW = 1024                   # col group width
NG = N // GW                # 4
K3 = C * 3 + 1              # 31 logical rows of the tp-matmul (single bf16)
KP = 34                     # partitions of the packed tp/sq tensor
RCLAMP = 1.0
EPS_D = 0.01   # rsqrt bias: diagonal d2 cancels to ~0 +- 3e-5 fp noise, so
               # +0.01 keeps r_diag <= ~10 (then min-clamped); off-diagonal
               # d2 >= ~120 so the bias error is <= 4e-5 relative
RSQRT_WIDE = False   # single 1024-wide ACT rsqrt vs 2x512 (PSUM bank reads)

f32 = mybir.dt.float32
f16 = mybir.dt.float16
bf16 = mybir.dt.bfloat16
NPBF16 = mybir.dt.np(bf16)

_CACHE = {}

# exposed for test.py
LAST_RESULTS = None


def _build_nc(seg_layout):
    """seg_layout: tuple per group of ((start, end, segidx), ...) column
    segments (class runs clipped to the group)."""
    nseg = 1 + max(s[2] for g in seg_layout for s in g)
    ocols = nseg + NG                       # w1 segments | w2 per group
    nc = bacc.Bacc("TRN2", target_bir_lowering=False, debug=False)

    rhs_g = nc.dram_tensor("rhs_g", [D, N], bf16, kind="ExternalInput")
    lhs_g = nc.dram_tensor("lhs_g", [D, ROWS], bf16, kind="ExternalInput")
    rhs3a = nc.dram_tensor("rhs3a", [KP, N], bf16, kind="ExternalInput")
    lhs3 = nc.dram_tensor("lhs3", [KP, ROWS], bf16, kind="ExternalInput")
    biases = nc.dram_tensor("biases", [128, CHUNKS], f32, kind="ExternalInput")
    out_d = nc.dram_tensor("out", [128, ocols * CHUNKS], f32, kind="ExternalOutput")

    with ExitStack() as ctx:
        tc = ctx.enter_context(tile.TileContext(nc))
        singles = ctx.enter_context(tc.tile_pool(name="singles", bufs=1))
        pd2 = ctx.enter_context(tc.tile_pool(name="pd2", bufs=2, space="PSUM"))
        ptp = ctx.enter_context(tc.tile_pool(name="ptp", bufs=2, space="PSUM"))
        work = ctx.enter_context(tc.tile_pool(name="work", bufs=4))
        stats = ctx.enter_context(tc.tile_pool(name="stats", bufs=4))

        # ---- load inputs ----
        # group-0 dependencies first, strictly in consumption order; strips
        # split between the sync and gpsimd queues so transfers overlap.
        # lhs3/rhs3a pack: rows 0-30 = tp weights, rows 32-33 = ones / sq
        # hi-lo rows (consumed at PE tile_position (32,0)).
        sb_lhsg = singles.tile([D, ROWS], bf16)
        sb_l3 = singles.tile([KP, ROWS], bf16)
        sb_bias = singles.tile([128, CHUNKS], f32)
        sb_rhsg = [
            singles.tile([D, GW], bf16, tag=f"rhsg{s}", name=f"rhsg{s}")
            for s in range(NG)
        ]
        sb_r3a = [
            singles.tile([KP, GW], bf16, tag=f"r3a{s}", name=f"r3a{s}")
            for s in range(NG)
        ]
        nc.sync.dma_start(out=sb_lhsg, in_=lhs_g[:, :])
        nc.sync.dma_start(out=sb_rhsg[0], in_=rhs_g[:, 0:GW])
        nc.gpsimd.dma_start(out=sb_l3, in_=lhs3[:, :])
        nc.gpsimd.dma_start(out=sb_r3a[0], in_=rhs3a[:, 0:GW])
        nc.gpsimd.dma_start(out=sb_bias, in_=biases[:, :])
        for s in range(1, NG):
            ssl = slice(s * GW, (s + 1) * GW)
            nc.sync.dma_start(out=sb_rhsg[s], in_=rhs_g[:, ssl])
            nc.gpsimd.dma_start(out=sb_r3a[s], in_=rhs3a[:, ssl])

        for c in range(CHUNKS):
            csl = slice(c * 128, (c + 1) * 128)
            st = stats.tile([128, ocols], f32, tag="st")
            for g in range(NG):
                d2t = pd2.tile([128, GW], f32, tag="d2", name=f"d2_{c}_{g}")
                tp = ptp.tile([128, GW], f32, tag="tp", name=f"tp_{c}_{g}")
                for h in (0, 1):
                    hs = slice(h * 512, (h + 1) * 512)
                    nc.tensor.matmul(
                        d2t[:, hs], lhsT=sb_lhsg[:, csl],
                        rhs=sb_rhsg[g][:, hs], start=True, stop=False,
                        skip_group_check=True,
                    )
                # tp (K=31, PE rows 0-31) and sq (K=2, PE rows 32-33) run
                # concurrently on different row-groups
                for h in (0, 1):
                    hs = slice(h * 512, (h + 1) * 512)
                    nc.tensor.matmul(
                        tp[:, hs], lhsT=sb_l3[0:K3, csl],
                        rhs=sb_r3a[g][0:K3, hs], start=True, stop=True,
                    )
                    nc.tensor.matmul(
                        d2t[:, hs], lhsT=sb_l3[32:34, csl],
                        rhs=sb_r3a[g][32:34, hs], start=False, stop=True,
                        tile_position=(32, 0), skip_group_check=True,
                    )
                r = work.tile([128, GW], f16, tag="r")
                if RSQRT_WIDE:
                    nc.scalar.activation(
                        r, d2t,
                        mybir.ActivationFunctionType.Abs_reciprocal_sqrt,
                        bias=sb_bias[:, c:c + 1], scale=1.0,
                    )
                else:
                    for h in (0, 1):
                        hs = slice(h * 512, (h + 1) * 512)
                        nc.scalar.activation(
                            r[:, hs], d2t[:, hs],
                            mybir.ActivationFunctionType.Abs_reciprocal_sqrt,
                            bias=sb_bias[:, c:c + 1], scale=1.0,
                        )
                q = work.tile([128, GW], f16, tag="q")
                for (s, e, sidx) in seg_layout[g]:
                    ls = slice(s - g * GW, e - g * GW)
                    nc.vector.scalar_tensor_tensor(
                        out=q[:, ls], in0=r[:, ls], scalar=RCLAMP,
                        in1=tp[:, ls], op0=mybir.AluOpType.min,
                        op1=mybir.AluOpType.mult,
                        accum_out=st[:, sidx:sidx + 1],
                    )
                junk = work.tile([128, GW], f16, tag="junk")
                w2col = st[:, nseg + g:nseg + g + 1]
                if (c * NG + g) % 2 == 0:
                    nc.scalar.activation(
                        junk, q, mybir.ActivationFunctionType.Square,
                        accum_out=w2col,
                    )
                else:
                    nc.vector.scalar_tensor_tensor(
                        out=junk, in0=q, scalar=1.0, in1=q,
                        op0=mybir.AluOpType.mult, op1=mybir.AluOpType.mult,
                        accum_out=w2col,
                    )
            nc.sync.dma_start(out=out_d[:, c * ocols:(c + 1) * ocols], in_=st)

    nc.compile()
    return nc, nseg, ocols


def _bf_split(x):
    """x (float64/32) -> (hi, lo) bf16 arrays with hi + lo ~= x."""
    x32 = np.asarray(x, np.float32)
    hi = x32.astype(NPBF16)
    lo = (x32 - hi.astype(np.float32)).astype(NPBF16)
    return hi, lo


def _segments(counts):
    """Class runs in sorted-column order, clipped to GW-wide groups.
    Returns (seg_layout, seg_class): per-group tuples of (s, e, segidx)
    and the class id of each segidx."""
    bounds = np.concatenate([[0], np.cumsum(counts)])
    seg_layout = []
    seg_class = []
    sidx = 0
    for g in range(NG):
        g0, g1 = g * GW, (g + 1) * GW
        segs = []
        for j in range(C):
            s, e = max(bounds[j], g0), min(bounds[j + 1], g1)
            if s < e:
                segs.append((int(s), int(e), sidx))
                seg_class.append(j)
                sidx += 1
        seg_layout.append(tuple(segs))
    return tuple(seg_layout), np.asarray(seg_class)


def _prep_inputs(pred, target, W):
    pred = np.asarray(pred, dtype=np.float32)
    target = np.asarray(target).astype(np.int64)
    W = np.asarray(W, dtype=np.float32)

    perm = np.argsort(target, kind="stable")
    pred = pred[perm]
    target = target[perm]

    p64 = pred.astype(np.float64)
    sq = (p64 * p64).sum(1)                                   # [N]
    wn = np.maximum(np.sqrt((W.astype(np.float64) ** 2).sum(1)), 1e-8)
    projn = (p64 @ W.T.astype(np.float64)) / wn[None, :]      # [N, C]
    counts = np.bincount(target, minlength=C)
    scb = 1.0 / np.sqrt(np.maximum(counts, 1))                # [C]
    colw = scb[target]                                        # [N]
    u = projn[np.arange(N), target]                           # [N]
    onehot = (target[:, None] == np.arange(C)[None, :]).astype(np.float64)

    gh, _ = _bf_split(-2.0 * pred.T)                          # hi only for G
    sqh, sql = _bf_split(sq)

    m = np.empty((K3, N), dtype=np.float64)
    m[0:C] = colw[None, :] * projn.T
    m[C] = -colw
    m[C + 1:2 * C + 1] = -(colw[None, :] * (onehot * projn).T)
    m[2 * C + 1:3 * C + 1] = colw[None, :] * onehot.T
    r3a_full = np.zeros((KP, N), dtype=NPBF16)
    r3a_full[0:K3] = m.astype(np.float32).astype(NPBF16)
    r3a_full[32] = sqh
    r3a_full[33] = sql

    l3_64 = np.empty((K3, N), dtype=np.float64)
    l3_64[0:C] = onehot.T
    l3_64[C] = u
    l3_64[C + 1:2 * C + 1] = onehot.T
    l3_64[2 * C + 1:3 * C + 1] = (u[:, None] * onehot).T
    l3_full = np.zeros((KP, N), dtype=NPBF16)
    l3_full[0:K3] = l3_64.astype(np.float32).astype(NPBF16)
    l3_full[32:34] = 1.0

    in_maps = []
    for k in range(NCORES):
        rs = slice(k * ROWS, (k + 1) * ROWS)
        sq_own = sq[rs].reshape(CHUNKS, 128).T                # [128, CHUNKS]
        in_maps.append(
            {
                "rhs_g": np.ascontiguousarray(gh),
                "lhs_g": np.ascontiguousarray(pred.T[:, rs].astype(NPBF16)),
                "rhs3a": np.ascontiguousarray(r3a_full),
                "lhs3": np.ascontiguousarray(l3_full[:, rs]),
                "biases": (sq_own + EPS_D).astype(np.float32),
            }
        )
    aux = {"counts": counts, "target": target, "scb": scb}
    return in_maps, aux


def _finish(per_core_out, aux, nseg, ocols, seg_class):
    counts, target, scb = aux["counts"], aux["target"], aux["scb"]
    cb = 1.0 / np.maximum(counts, 1)
    cb_a = cb[target]                                         # [N]
    mu = scb[seg_class]                                       # per-segment factor
    S = np.zeros((C, 2), dtype=np.float64)
    for k, o in enumerate(per_core_out):
        o = o.astype(np.float64)
        for c in range(CHUNKS):
            rs = slice(k * ROWS + c * 128, k * ROWS + (c + 1) * 128)
            oc = o[:, c * ocols:(c + 1) * ocols]
            inner1 = oc[:, :nseg] @ mu                        # [128]
            w2row = oc[:, nseg:nseg + NG].sum(1)              # [128]
            wrow = cb_a[rs]                                   # [128]
            tcls = target[rs]
            np.add.at(S[:, 0], tcls, wrow * inner1)
            np.add.at(S[:, 1], tcls, wrow * w2row)
    exist = float((counts > 0).sum())
    valid = counts > 0
    W1 = -S[:, 0]
    W2 = S[:, 1]
    W0 = exist - 1.0
    denom = exist - 1.0
    l1 = (W0 - 2.0 * W1 + W2) / denom
    mmn = W1 / denom
    mv = (W2 - 2.0 * mmn * W1 + mmn * mmn * W0) / denom
    safe_mm = np.where(mmn == 0.0, 1.0, mmn)
    loss1 = float(np.where(valid, l1, 0.0).sum() / exist)
    loss2 = float(np.where(valid, np.abs(mv / safe_mm), 0.0).sum() / exist)
    return (
        np.asarray(loss1, dtype=np.float32),
        np.asarray(loss2, dtype=np.float32),
    )


def kernel(pred, target, W):
    global LAST_RESULTS
    in_maps, aux = _prep_inputs(pred, target, W)
    seg_layout, seg_class = _segments(aux["counts"])
    if seg_layout not in _CACHE:
        _CACHE[seg_layout] = _build_nc(seg_layout)
    nc, nseg, ocols = _CACHE[seg_layout]
    res = run_bass_kernel_spmd(nc, in_maps, list(range(NCORES)))
    LAST_RESULTS = res
    per_core = [res.results[k]["out"] for k in range(NCORES)]
    return _finish(per_core, aux, nseg, ocols, seg_class)
